# revision 14
# baseline (speedup 1.0000x reference)
"""CachedGCN (2-layer GCN, shared sparse A) on 8 Trainium2 NeuronCores.

Math:  out = A @ relu((A @ X) @ W1) @ W3
Rewritten as:  z = A @ X ; g = relu(z @ W1) @ W3 ; out = A @ g
so the inter-layer exchange (AllGather) moves the 64-dim g instead of the
128-dim hidden h, and both SpMMs gather 256-byte rows.

Distribution: 1-D graph partition. Each core owns 12544 destination rows
(8 x 12544 = 100352 >= 100000). Edges are bucketed by destination-row
group (64 rows) and source-column chunk (4 chunks of 25088 nodes, so
within-chunk indices fit int16 for dma_gather). Per (group, chunk) cell the
edge list is padded to a multiple of 128 (cross-core max, so the SPMD
program is identical on every core).

Per 128-edge stripe the kernel:
  - builds sel[p, r] = val[p] * (row_in_group[p] == r) with one fused
    DVE tensor_scalar (iota is_equal rowloc) * val
  - matmuls psum_zT[64d, 64r] += gathered[128e, 64d].T @ sel[128e, 64r]
accumulating over each group's stripes in PSUM.  Source rows are fetched
with dma_gather (Q7 SWDGE custom gather, int16 indices, interleaved
16-partition wrap).  zT feeds dense W1/W3 matmuls; g is transposed to
node-major, AllGathered across the 8 cores, and the second SpMM runs the
same edge structure against g.
"""

import os
import sys
import numpy as np

sys.path.insert(0, "/opt/trn_rl_repo")

import concourse.bass as bass
import concourse.bacc as bacc
import concourse.mybir as mybir
import concourse.tile as tile
from concourse.vector_clock import ScopedClock

# ----------------------------------------------------------------------------
# Environment compatibility patches
# ----------------------------------------------------------------------------

_PATCHED = False


def _install_patches():
    """The walrus build in this env caps CTRL instructions (Drain/Branch) at
    ONE sync-wait command (a sem-eq wait counts as two).  Tile's tail barrier
    emits multi-wait drains; replace with sem-only barriers + one-wait nops."""
    global _PATCHED
    if _PATCHED:
        return
    _PATCHED = True

    _orig_aeb = bass.Bass.all_engine_barrier

    def _aeb(self, *, sem_only=False):
        return _orig_aeb(self, sem_only=True)

    def _drain_and_barrier(self, tick_clock, wait_clock):
        nc = self.nc
        drain_inst = nc.sync.drain()
        wait_clock.add_sem_waits(
            drain_inst.ins, ScopedClock({None: tick_clock.global_clock})
        )
        si = drain_inst.ins.sync_info
        if si is not None and si.on_wait is not None and len(si.on_wait) > 1:
            waits = list(si.on_wait)
            drain_inst.ins.sync_info = mybir.SyncInfo(
                on_wait=[waits[0]], on_update=list(si.on_update or [])
            )
            for w in waits[1:]:
                nop = nc.sync.nop()
                nop.ins.sync_info = mybir.SyncInfo(on_wait=[w], on_update=[])
        nc.all_engine_barrier()
        assert self.sems is not None
        popped = nc._tile_sem_poison_stack.pop()
        assert popped is self._sem_poison
        nc.clear_and_free_semaphores(list(self.sems.allocated().values()))
        nc.all_engine_barrier()

    bass.Bass.all_engine_barrier = _aeb
    tile.TileContext._drain_and_barrier = _drain_and_barrier


def _install_profile_hook():
    """Register the axon NTFF profiling hook (for trace=True) if available."""
    import types

    if "antenv.axon_hooks" in sys.modules:
        return
    try:
        import antenv
        from trn_agent_boot.trn_boot import _ntff_profile_via_ctypes

        hook = _ntff_profile_via_ctypes("/opt/axon/libaxon_pjrt.so")
    except Exception:
        return
    mod = types.ModuleType("antenv.axon_hooks")
    _state = {"hook": hook}
    mod.set_axon_ntff_profile_hook = lambda h: _state.__setitem__("hook", h)
    mod.get_axon_ntff_profile_hook = lambda: _state["hook"]
    sys.modules["antenv.axon_hooks"] = mod
    antenv.axon_hooks = mod


# ----------------------------------------------------------------------------
# Problem constants (hardcoded per the harness contract)
# ----------------------------------------------------------------------------

N_NODES = 100000
N_EDGES = 1600000
DIN = 64
HID = 128
DOUT = 64

P = 128          # partitions / edges per stripe
G = 64           # destination rows per group (= sel width)
R_SHARD = 12544  # rows per core (196 groups)
NG = R_SHARD // G          # 196
SG = 7                     # groups per super-group (psum region 7*64=448 f32)
NSG = NG // SG             # 28
N_PAD = 8 * R_SHARD        # 100352
NCH = 4
CH = N_PAD // NCH          # 25088 (< 32768 so int16 indices work)

F32 = mybir.dt.float32
I16 = mybir.dt.int16

LAST_RUN_INFO = {}


# ----------------------------------------------------------------------------
# Host-side preprocessing: bucket, pad and order the edge list
# ----------------------------------------------------------------------------

def _preprocess(edge_row, edge_col, edge_val):
    r = np.asarray(edge_row).astype(np.int64)
    c = np.asarray(edge_col).astype(np.int64)
    v = np.asarray(edge_val).astype(np.float32)

    core = r // R_SHARD
    rl = r - core * R_SHARD
    g = rl // G
    rig = (rl - g * G).astype(np.float32)   # row-in-group, exact in f32
    ch = c // CH
    cin = (c - ch * CH).astype(np.int16)    # within-chunk source index

    # per-(core, group, chunk) counts -> per-cell stripe count (cross-core max)
    cell = (core * NG + g) * NCH + ch
    counts = np.bincount(cell, minlength=8 * NG * NCH).reshape(8, NG, NCH)
    T = -(-counts.max(axis=0) // P)          # [NG, NCH] stripes per cell

    # padded stream layout, identical on every core:
    # for sg: for ch: for g in sg-groups: T[g,ch]*128 edge slots
    cell_off = np.zeros((NG, NCH), dtype=np.int64)
    call_meta = []  # [sg][ch] -> dict(num, idx_off(cols))
    group_stripes = {}  # (sg, g_in_sg) -> ordered [(ch, slot_in_call, s)]
    off = 0
    for sgi in range(NSG):
        row_of_calls = []
        for chi in range(NCH):
            call_num = 0
            call_off = off
            for gi in range(sgi * SG, (sgi + 1) * SG):
                cell_off[gi, chi] = off
                for _ in range(int(T[gi, chi])):
                    group_stripes.setdefault((sgi, gi - sgi * SG), []).append(
                        (chi, (off - call_off) // P, off // P)
                    )
                    off += P
                call_num += int(T[gi, chi]) * P
            row_of_calls.append(dict(num=call_num, idx_off=call_off // 16))
        call_meta.append(row_of_calls)
    e_pad = off
    n_stripe = e_pad // P

    # scatter the edges of each core into the padded stream
    idx16 = np.zeros((8, e_pad), dtype=np.int16)
    rowloc = np.zeros((8, e_pad), dtype=np.float32)
    vals = np.zeros((8, e_pad), dtype=np.float32)

    # position of each edge: cell_off[g,ch] + within-cell rank (per core)
    flat_cell = (g * NCH + ch)
    enc = core * (NG * NCH) + flat_cell
    order = np.argsort(enc, kind="stable")
    enc_sorted = enc[order]
    # rank within equal enc runs
    run_start = np.r_[0, np.flatnonzero(np.diff(enc_sorted)) + 1]
    ranks = np.arange(len(enc_sorted)) - np.repeat(run_start, np.diff(np.r_[run_start, len(enc_sorted)]))
    pos_sorted = cell_off[g[order], ch[order]] + ranks
    core_sorted = core[order]
    idx16[core_sorted, pos_sorted] = cin[order]
    rowloc[core_sorted, pos_sorted] = rig[order]
    vals[core_sorted, pos_sorted] = v[order]

    # idx16 interleaved wrap: seq[j] -> arr[j % 16, j // 16], per call slice is
    # contiguous in columns because call lengths are multiples of 128.
    idx_wrapped = np.ascontiguousarray(
        idx16.reshape(8, e_pad // 16, 16).transpose(0, 2, 1)
    )  # [8, 16, e_pad/16]
    idx_tile = np.tile(idx_wrapped, (1, 8, 1))  # replicate to 128 partitions

    rowloc_t = np.ascontiguousarray(rowloc.reshape(8, n_stripe, P).transpose(0, 2, 1))
    vals_t = np.ascontiguousarray(vals.reshape(8, n_stripe, P).transpose(0, 2, 1))

    meta = dict(
        e_pad=e_pad,
        n_stripe=n_stripe,
        call_meta=call_meta,
        group_stripes=group_stripes,
    )
    return meta, idx_tile, rowloc_t, vals_t


# ----------------------------------------------------------------------------
# Device program
# ----------------------------------------------------------------------------

def _build_program(meta, debug=False, stage="full"):
    nc = bacc.Bacc(None)

    feat = nc.dram_tensor("feat", [N_PAD, DIN], F32, kind="ExternalInput")
    idx16 = nc.dram_tensor("idx16", [P, meta["e_pad"] // 16], I16, kind="ExternalInput")
    rowloc = nc.dram_tensor("rowloc", [P, meta["n_stripe"]], F32, kind="ExternalInput")
    vals = nc.dram_tensor("vals", [P, meta["n_stripe"]], F32, kind="ExternalInput")
    iota = nc.dram_tensor("iota", [P, G], F32, kind="ExternalInput")
    iden = nc.dram_tensor("iden", [P, P], F32, kind="ExternalInput")
    w1 = nc.dram_tensor("w1", [DIN, HID], F32, kind="ExternalInput")
    w3 = nc.dram_tensor("w3", [HID, DOUT], F32, kind="ExternalInput")
    outp = nc.dram_tensor("outp", [R_SHARD, DOUT], F32, kind="ExternalOutput")
    if debug:
        z_dbg = nc.dram_tensor("z_dbg", [DIN, R_SHARD], F32, kind="ExternalOutput")
        g_dbg = nc.dram_tensor("g_dbg", [R_SHARD, DOUT], F32, kind="ExternalOutput")

    call_meta = meta["call_meta"]
    group_stripes = meta["group_stripes"]

    with tile.TileContext(nc) as tc:
        with (
            tc.tile_pool(name="const", bufs=1) as constp,
            tc.tile_pool(name="big", bufs=1) as bigp,
            tc.tile_pool(name="gath", bufs=8) as gathp,
            tc.tile_pool(name="sel", bufs=8) as selp,
            tc.tile_pool(name="work", bufs=2) as workp,
            tc.tile_pool(name="psz", bufs=2, space="PSUM") as pszp,
            tc.tile_pool(name="psd", bufs=2, space="PSUM") as psdp,
            tc.tile_pool(name="dram", bufs=1, space="DRAM") as dramp,
        ):
            # resident data
            idx_sb = bigp.tile([P, meta["e_pad"] // 16], I16)
            nc.sync.dma_start(out=idx_sb[:], in_=idx16[:])
            rowloc_sb = bigp.tile([P, meta["n_stripe"]], F32)
            nc.sync.dma_start(out=rowloc_sb[:], in_=rowloc[:])
            vals_sb = bigp.tile([P, meta["n_stripe"]], F32)
            nc.sync.dma_start(out=vals_sb[:], in_=vals[:])
            iota_sb = constp.tile([P, G], F32)
            nc.sync.dma_start(out=iota_sb[:], in_=iota[:])
            iden_sb = constp.tile([P, P], F32)
            nc.sync.dma_start(out=iden_sb[:], in_=iden[:])
            w1_sb = constp.tile([DIN, HID], F32)
            nc.sync.dma_start(out=w1_sb[:], in_=w1[:])
            w3_sb = constp.tile([HID, DOUT], F32)
            nc.sync.dma_start(out=w3_sb[:], in_=w3[:])
            zero_sb = constp.tile([P, G], F32)
            nc.vector.memset(zero_sb[:], 0.0)

            zt_sb = bigp.tile([DIN, R_SHARD], F32)     # zT then reused as outT
            g_nm = dramp.tile([R_SHARD, DOUT], F32)    # node-major g shard
            g_full = dramp.tile([N_PAD, DOUT], F32, addr_space="Shared")

            def spmm(src_ap, dst_sb, phase):
                """dst_sb[64, R_SHARD] = (A_core @ src).T via gather+sel matmuls.

                start=True invalidates the WHOLE psum bank on this HW, so each
                group's accumulation chain runs sequentially in its own
                [64, 64] psum tile (alternating banks via the pool)."""
                for sgi in range(NSG):
                    gbufs = {}
                    for chi in range(NCH):
                        cm = call_meta[sgi][chi]
                        if cm["num"] == 0:
                            continue
                        slots = cm["num"] // P
                        gbuf = gathp.tile([P, slots, DIN], F32,
                                          name=f"gb{phase}_{sgi}_{chi}", tag="gb")
                        nc.gpsimd.dma_gather(
                            out_ap=gbuf[:],
                            in_ap=src_ap[chi * CH:(chi + 1) * CH, :],
                            idxs_ap=idx_sb[:, cm["idx_off"]: cm["idx_off"] + cm["num"] // 16],
                            num_idxs=cm["num"],
                            num_idxs_reg=cm["num"],
                            elem_size=DIN,
                            single_packet=False,
                        )
                        gbufs[chi] = gbuf
                    for gis in range(SG):
                        stripes = group_stripes.get((sgi, gis), [])
                        psz = pszp.tile([DIN, G], F32,
                                        name=f"psz{phase}_{sgi}_{gis}", tag="psz")
                        if not stripes:  # group with zero edges
                            nc.tensor.matmul(psz[:], lhsT=zero_sb[:],
                                             rhs=zero_sb[:], start=True, stop=True)
                        for k, (chi, j, s) in enumerate(stripes):
                            sel = selp.tile([P, G], F32,
                                            name=f"sel{phase}_{sgi}_{gis}_{k}",
                                            tag="sel")
                            nc.vector.tensor_scalar(
                                out=sel[:],
                                in0=iota_sb[:],
                                scalar1=rowloc_sb[:, s:s + 1],
                                scalar2=vals_sb[:, s:s + 1],
                                op0=mybir.AluOpType.is_equal,
                                op1=mybir.AluOpType.mult,
                            )
                            nc.tensor.matmul(
                                psz[:],
                                lhsT=gbufs[chi][:, j, :],
                                rhs=sel[:],
                                start=(k == 0),
                                stop=(k == len(stripes) - 1),
                            )
                        nc.scalar.copy(
                            out=dst_sb[:, (sgi * SG + gis) * G:(sgi * SG + gis + 1) * G],
                            in_=psz[:],
                        )

            # ---- layer 1 SpMM: zT = (A @ X).T
            spmm(feat[:, :], zt_sb, 0)
            if debug:
                nc.sync.dma_start(out=z_dbg[:], in_=zt_sb[:])

            if stage != "z":
                # ---- dense: g = relu(z @ W1) @ W3, node-major into g_nm
                NT = 512
                for nb in range((R_SHARD + NT - 1) // NT):
                    o = nb * NT
                    nt = min(NT, R_SHARD - o)
                    psh = psdp.tile([HID, NT], F32, name=f"psh{nb}", tag="psh")
                    nc.tensor.matmul(psh[:, :nt], lhsT=w1_sb[:],
                                     rhs=zt_sb[:, o:o + nt], start=True, stop=True)
                    ht = workp.tile([HID, NT], F32, name=f"ht{nb}", tag="ht")
                    nc.vector.tensor_relu(out=ht[:, :nt], in_=psh[:, :nt])
                    psg = psdp.tile([DOUT, NT], F32, name=f"psg{nb}", tag="psg")
                    nc.tensor.matmul(psg[:, :nt], lhsT=w3_sb[:], rhs=ht[:, :nt],
                                     start=True, stop=True)
                    gt = workp.tile([DOUT, NT], F32, name=f"gt{nb}", tag="gt")
                    nc.scalar.copy(out=gt[:, :nt], in_=psg[:, :nt])
                    # transpose 128-col blocks to node-major and store
                    for b in range(0, nt, P):
                        pst = pszp.tile([P, DOUT], F32, name=f"pst{nb}_{b}",
                                        tag="pst")
                        nc.tensor.transpose(
                            out=pst[:], in_=gt[:, b:b + P],
                            identity=iden_sb[:DOUT, :DOUT]
                        )
                        gsb = workp.tile([P, DOUT], F32, name=f"gsb{nb}_{b}",
                                         tag="gsb")
                        nc.vector.tensor_copy(out=gsb[:], in_=pst[:])
                        nc.sync.dma_start(out=g_nm[o + b:o + b + P, :], in_=gsb[:])

                if debug:
                    nc.sync.dma_start(out=g_dbg[:], in_=g_nm[:])

            if stage == "full":
                # ---- AllGather g shards -> g_full [100352, 64]
                nc.gpsimd.collective_compute(
                    "AllGather",
                    mybir.AluOpType.bypass,
                    replica_groups=[list(range(8))],
                    ins=[g_nm[:]],
                    outs=[g_full[:]],
                )

                # ---- layer 2 SpMM: outT = (A @ g).T  (reuse zt_sb)
                spmm(g_full[:, :], zt_sb, 1)

            # ---- transpose (stage=="full": outT, else zT) -> node-major output
            for b in range(0, R_SHARD, P):
                pst = pszp.tile([P, DOUT], F32, name=f"psto{b}", tag="pst")
                nc.tensor.transpose(
                    out=pst[:], in_=zt_sb[:, b:b + P], identity=iden_sb[:DOUT, :DOUT]
                )
                osb = workp.tile([P, DOUT], F32, name=f"osb{b}", tag="gsb")
                nc.vector.tensor_copy(out=osb[:], in_=pst[:])
                nc.sync.dma_start(out=outp[b:b + P, :], in_=osb[:])

    nc.finalize()
    return nc


# ----------------------------------------------------------------------------
# Entry point
# ----------------------------------------------------------------------------

def kernel(features, edge_row, edge_col, edge_val, weight1, weight3):
    _install_patches()
    trace = bool(int(os.environ.get("GCN_TRACE", "0")))
    debug = bool(int(os.environ.get("GCN_DEBUG", "0")))
    if trace:
        _install_profile_hook()

    from concourse.bass_utils import run_bass_kernel_spmd

    stage = os.environ.get("GCN_STAGE", "full")
    meta, idx_tile, rowloc_t, vals_t = _preprocess(edge_row, edge_col, edge_val)
    nc = _build_program(meta, debug=debug, stage=stage)

    feat_pad = np.zeros((N_PAD, DIN), np.float32)
    feat_pad[:N_NODES] = np.asarray(features, np.float32)
    iota_v = np.tile(np.arange(G, dtype=np.float32), (P, 1))
    iden_v = np.eye(P, dtype=np.float32)
    w1_v = np.asarray(weight1, np.float32)
    w3_v = np.asarray(weight3, np.float32)

    in_maps = [
        dict(
            feat=feat_pad,
            idx16=np.ascontiguousarray(idx_tile[c]),
            rowloc=np.ascontiguousarray(rowloc_t[c]),
            vals=np.ascontiguousarray(vals_t[c]),
            iota=iota_v,
            iden=iden_v,
            w1=w1_v,
            w3=w3_v,
        )
        for c in range(8)
    ]

    res = run_bass_kernel_spmd(nc, in_maps, core_ids=list(range(8)), trace=trace)
    LAST_RUN_INFO["exec_time_ns"] = res.exec_time_ns
    LAST_RUN_INFO["results"] = res.results if debug else None
    LAST_RUN_INFO["meta"] = meta

    out = np.concatenate([res.results[c]["outp"] for c in range(8)], axis=0)
    return np.ascontiguousarray(out[:N_NODES])
